# revision 1
# baseline (speedup 1.0000x reference)
"""YOLO-v1-style loss on 8 Trainium2 NeuronCores (Bass/Tile).

Data-parallel over batch: each core gets 2048 of 16384 batch elements
([2048,7,7,30] -> 128 partitions x 784 cells x 30 channels), computes
per-partition partial sums for the 5 loss terms on-device, host combines.

Inputs are converted to bf16 on the host: halves DMA traffic and enables
the DVE 2x perf mode on contiguous tensor_tensor ops. The resulting
relative error on each loss term is ~1e-5 (sums over millions of terms:
rounding noise averages out; the systematic bias is ~(p^2+t^2)*var(eps)).

Self-contained: hardcodes all shapes; only needs numpy + concourse (bass).
"""

import numpy as np
import ml_dtypes

import concourse.bass as bass
import concourse.bacc as bacc
import concourse.tile as tile
import concourse.mybir as mybir
from concourse.bass_utils import run_bass_kernel_spmd

f32 = mybir.dt.float32
bf16 = mybir.dt.bfloat16
Alu = mybir.AluOpType
Act = mybir.ActivationFunctionType
X = mybir.AxisListType.X

S = 7
B = 2
D = 30
BATCH = 16384
NCORES = 8
PER = BATCH // NCORES          # 2048 batch elems per core
P = 128                        # partitions
F = PER * S * S // P           # 784 cells per partition
NCHUNK = 4
CH = F // NCHUNK               # cells per partition per chunk
NACC = NCHUNK * 5              # accumulator columns (5 terms per chunk)

INV_S = 1.0 / S


def _bc_box(x):
    """[P, CH, ...] -> [P, 2, CH, ...]: broadcast over the box dim (step 0)."""
    return bass.AP(tensor=x.tensor, offset=x.offset,
                   ap=[x.ap[0], [0, 2]] + list(x.ap[1:]))


def _flat2(x, n):
    """Contiguous [P, 2, ch, 2] tile -> 2-free-dim view [[2, n], [1, 2]]."""
    return bass.AP(tensor=x.tensor, offset=x.offset,
                   ap=[x.ap[0], [2, n], [1, 2]])


def _bc_pair(x, n):
    """Contiguous [P, 2, ch] tile -> [[1, n], [0, 2]] (repeat each value 2x)."""
    return bass.AP(tensor=x.tensor, offset=x.offset,
                   ap=[x.ap[0], [1, n], [0, 2]])


def build_nc(f=F, nchunk=NCHUNK, repeat=1, variant="full"):
    ch = f // nchunk
    nacc = nchunk * 5
    nc = bacc.Bacc("TRN2", target_bir_lowering=False, debug=False,
                   num_devices=NCORES)
    # hybrid inputs, host-pre-split: box channels (0..9) f32 for exact
    # IoU/selection, class channels (10..29) bf16 (error averages out).
    box = nc.dram_tensor("box", [2, P, f, 10], f32, kind="ExternalInput")
    cls_ = nc.dram_tensor("cls", [2, P, f, 20], bf16, kind="ExternalInput")
    out = nc.dram_tensor("acc_out", [P, nacc], f32, kind="ExternalOutput")
    box_pm = box.ap().rearrange("two p f d -> p two f d")
    cls_pm = cls_.ap().rearrange("two p f d -> p two f d")

    V = nc.vector
    A = nc.scalar
    G = nc.gpsimd

    with tile.TileContext(nc) as tc:
        with (
            tc.tile_pool(name="inp", bufs=2) as inp,
            tc.tile_pool(name="wk2", bufs=2) as wk2,
            tc.tile_pool(name="wk1", bufs=1) as wk1,
            tc.tile_pool(name="one", bufs=1) as one,
        ):
            acc = one.tile([P, nacc], f32)
            V.memset(acc, 0.0)

            for k in range(nchunk * repeat):
                k = k % nchunk
                c0 = k * ch

                boxt = inp.tile([P, 2, ch, 10], f32, tag="boxt")
                nc.sync.dma_start(boxt, box_pm[:, :, c0:c0 + ch, :])
                clst = inp.tile([P, 2, ch, 20], bf16, tag="clst")
                nc.sync.dma_start(clst, cls_pm[:, :, c0:c0 + ch, :])

                # box-major views [P, 2, ch, 5]
                pb = boxt[:, 0].rearrange("p c (b k) -> p b c k", b=2)
                tb = boxt[:, 1].rearrange("p c (b k) -> p b c k", b=2)
                pxyr = pb[:, :, :, 0:2]
                pwhr = pb[:, :, :, 2:4]
                pcfr = pb[:, :, :, 4]      # [P,2,ch] conf ch 4,9
                twhr = tb[:, :, :, 2:4]
                tcfr = tb[:, :, :, 4]
                txyr = tb[:, :, :, 0:2]
                t4 = tb[:, 0, :, 4]        # [P,ch] obj mask (exactly 0/1)

                obj = wk1.tile([P, ch], f32, tag="obj")
                A.activation(obj, t4, Act.Copy)
                if variant in ("full", "dve"):
                    # ---- ACT extractions ----
                    pwh = wk2.tile([P, 2, ch, 2], f32, tag="pwh")   # 0.5*w, 0.5*h
                    A.activation(pwh, pwhr, Act.Copy, scale=0.5)
                    pxy = wk2.tile([P, 2, ch, 2], f32, tag="pxy")   # x/S, y/S
                    A.activation(pxy, pxyr, Act.Copy, scale=INV_S)
                    twh = wk2.tile([P, 2, ch, 2], f32, tag="twh")
                    A.activation(twh, twhr, Act.Copy, scale=0.5)
                    txy = wk1.tile([P, ch, 2], f32, tag="txy")      # t box0
                    A.activation(txy, tb[:, 0, :, 0:2], Act.Copy, scale=INV_S)
                    noobjm = wk1.tile([P, ch], f32, tag="noobjm")
                    A.activation(noobjm, t4, Act.Copy, scale=-1.0, bias=1.0)
                    pconf = wk1.tile([P, 2, ch], f32, tag="pconf")
                    A.activation(pconf, pcfr, Act.Copy)

                    # ---- diffs from raw inputs (strided reads, 1x) ----
                    dxy = wk2.tile([P, 2, ch, 2], f32, tag="dxy")
                    V.tensor_tensor(dxy, pxyr, txyr, op=Alu.subtract)
                    d2xy = wk2.tile([P, 2, ch, 2], f32, tag="d2xy")
                    A.square(d2xy, dxy)
                    swh = wk2.tile([P, 2, ch, 2], f32, tag="swh")   # pw + tw
                    V.tensor_tensor(swh, pwhr, twhr, op=Alu.add)
                    qwh = wk2.tile([P, 2, ch, 2], f32, tag="qwh")   # (pw/2)(tw/2)
                    V.tensor_tensor(qwh, pwh, twh, op=Alu.mult)
                    rwh = wk2.tile([P, 2, ch, 2], f32, tag="rwh")    # 2*sqrt(pw*tw)
                    A.activation(rwh, qwh, Act.Sqrt, scale=16.0)
                    dconf = wk1.tile([P, 2, ch], f32, tag="dconf")
                    V.tensor_tensor(dconf, pcfr, tcfr, op=Alu.subtract)
                    A.square(dconf, dconf)                  # in-place -> d2conf
                if variant in ("full", "pool"):
                    # class diffs (bf16, on POOL)
                    dcls = wk2.tile([P, ch, 20], bf16, tag="dcls", bufs=2)
                    G.tensor_tensor(dcls, clst[:, 0], clst[:, 1],
                                    op=Alu.subtract)
                    # mask by obj on POOL, square+accumulate on ACT
                    dm = wk2.tile([P, ch, 20], bf16, tag="dm", bufs=1)
                    objbc = bass.AP(tensor=obj.tensor, offset=obj.offset,
                                    ap=[obj.ap[0], [1, ch], [0, 20]])
                    G.tensor_tensor(dm, dcls, objbc, op=Alu.mult)

                if variant in ("full", "dve"):
                    # ---- corners (bf16 contiguous, 2x) ----
                    pc1 = wk2.tile([P, 2, ch, 2], f32, tag="pc1")
                    V.tensor_tensor(pc1, pxy, pwh, op=Alu.subtract)
                    pc2 = wk2.tile([P, 2, ch, 2], f32, tag="pc2")
                    V.tensor_tensor(pc2, pxy, pwh, op=Alu.add)
                    tc1 = wk1.tile([P, ch, 2], f32, tag="tc1")
                    V.tensor_tensor(tc1, txy, twh[:, 0], op=Alu.subtract)
                    tc2 = wk1.tile([P, ch, 2], f32, tag="tc2")
                    V.tensor_tensor(tc2, txy, twh[:, 0], op=Alu.add)

                    # ---- IoU ----
                    lt = wk1.tile([P, 2, ch, 2], f32, tag="lt")
                    V.tensor_tensor(lt, pc1, _bc_box(tc1), op=Alu.max)
                    rb = wk1.tile([P, 2, ch, 2], f32, tag="rb")
                    V.tensor_tensor(rb, pc2, _bc_box(tc2), op=Alu.min)
                    whd = wk1.tile([P, 2, ch, 2], f32, tag="whd")
                    V.tensor_tensor(whd, rb, lt, op=Alu.subtract)
                    A.activation(whd, whd, Act.Relu)        # in-place clamp >= 0
                    inter = wk1.tile([P, 2, ch], f32, tag="inter")
                    V.tensor_tensor(inter, whd[:, :, :, 0], whd[:, :, :, 1],
                                    op=Alu.mult)
                    areap = wk1.tile([P, 2, ch], f32, tag="areap")
                    V.tensor_tensor(areap, pb[:, :, :, 2], pb[:, :, :, 3],
                                    op=Alu.mult)
                    areat = wk1.tile([P, ch], f32, tag="areat")
                    V.tensor_tensor(areat, tb[:, 0, :, 2], tb[:, 0, :, 3],
                                    op=Alu.mult)
                    denom = wk1.tile([P, 2, ch], f32, tag="denom")
                    V.tensor_tensor(denom, areap, _bc_box(areat), op=Alu.add)
                    V.tensor_tensor(denom, denom, inter, op=Alu.subtract)
                    rden = wk1.tile([P, 2, ch], f32, tag="rden")
                    V.reciprocal_approx_fast(rden, denom)
                    iou = wk1.tile([P, 2, ch], f32, tag="iou")
                    V.tensor_tensor(iou, inter, rden, op=Alu.mult)

                    # ---- responsibility selection ----
                    ge = wk1.tile([P, ch], f32, tag="ge")
                    V.tensor_tensor(ge, iou[:, 0], iou[:, 1], op=Alu.is_ge)
                    miou = wk1.tile([P, ch], f32, tag="miou")
                    V.tensor_tensor(miou, iou[:, 0], iou[:, 1], op=Alu.max)
                    resp = wk1.tile([P, 2, ch], f32, tag="resp")
                    V.tensor_tensor(resp[:, 0], ge, obj, op=Alu.mult)
                    V.tensor_tensor(resp[:, 1], obj, resp[:, 0], op=Alu.subtract)

                # ---- loss terms -> acc columns (stt with fused accum) ----
                a0 = k * 5
                n2 = 2 * ch
                if variant in ("full", "dve"):
                    scr = wk1.tile([P, 2, ch, 2], f32, tag="scr", bufs=2)
                    V.scalar_tensor_tensor(
                        _flat2(scr, n2), _flat2(d2xy, n2), 0.0, _bc_pair(resp, n2),
                        op0=Alu.bypass, op1=Alu.mult,
                        accum_out=acc[:, a0 + 0:a0 + 1])

                    term = wk1.tile([P, 2, ch, 2], f32, tag="scr", bufs=2)
                    V.tensor_tensor(term, swh, rwh, op=Alu.subtract)
                    scr = wk1.tile([P, 2, ch, 2], f32, tag="scr", bufs=2)
                    V.scalar_tensor_tensor(
                        _flat2(scr, n2), _flat2(term, n2), 0.0, _bc_pair(resp, n2),
                        op0=Alu.bypass, op1=Alu.mult,
                        accum_out=acc[:, a0 + 1:a0 + 2])

                    odiff = wk1.tile([P, 2, ch], f32, tag="odiff")
                    V.tensor_tensor(odiff, pconf, _bc_box(miou), op=Alu.subtract)
                    osq = wk1.tile([P, 2, ch], f32, tag="osq")
                    A.square(osq, odiff)
                    scr = wk1.tile([P, 2, ch, 2], f32, tag="scr", bufs=2)
                    V.scalar_tensor_tensor(
                        scr[:, :, :, 0], osq, 0.0, resp,
                        op0=Alu.bypass, op1=Alu.mult,
                        accum_out=acc[:, a0 + 2:a0 + 3])

                    nb = wk1.tile([P, ch], f32, tag="nb")
                    V.tensor_tensor(nb, dconf[:, 0], dconf[:, 1], op=Alu.add)
                    scr = wk1.tile([P, 2, ch, 2], f32, tag="scr", bufs=2)
                    V.scalar_tensor_tensor(
                        scr[:, 0, :, 0], nb, 0.0, noobjm,
                        op0=Alu.bypass, op1=Alu.mult,
                        accum_out=acc[:, a0 + 3:a0 + 4])

                if variant in ("full", "pool"):
                    # class loss: sum((obj*d)^2) fused on ACT
                    A.activation(dm, dm, Act.Square,
                                 accum_out=acc[:, a0 + 4:a0 + 5])

            nc.sync.dma_start(out.ap(), acc)

    nc.compile()
    return nc


_NC_CACHE = None


def _get_nc():
    global _NC_CACHE
    if _NC_CACHE is None:
        _NC_CACHE = build_nc()
    return _NC_CACHE


def shard_inputs(pred_tensor, target_tensor):
    """Full [16384,7,7,30] f32 inputs -> per-core hybrid box(f32)/cls(bf16)."""
    p = np.ascontiguousarray(pred_tensor, dtype=np.float32).reshape(NCORES, P, F, D)
    t = np.ascontiguousarray(target_tensor, dtype=np.float32).reshape(NCORES, P, F, D)
    box = np.empty((NCORES, 2, P, F, 10), dtype=np.float32)
    box[:, 0] = p[..., 0:10]
    box[:, 1] = t[..., 0:10]
    cls_ = np.empty((NCORES, 2, P, F, 20), dtype=ml_dtypes.bfloat16)
    cls_[:, 0] = p[..., 10:30]
    cls_[:, 1] = t[..., 10:30]
    return [{"box": box[c], "cls": cls_[c]} for c in range(NCORES)]


def combine(results):
    """Per-core acc_out [P, NACC] -> 5-tuple of loss scalars."""
    total = np.zeros(5, dtype=np.float64)
    for r in results:
        a = r["acc_out"].astype(np.float64).sum(axis=0)   # [NACC]
        total += a.reshape(NCHUNK, 5).sum(axis=0)
    total /= BATCH
    return tuple(np.float32(v) for v in total)


def kernel(pred_tensor, target_tensor):
    nc = _get_nc()
    in_maps = shard_inputs(pred_tensor, target_tensor)
    res = run_bass_kernel_spmd(nc, in_maps, core_ids=list(range(NCORES)))
    return combine(res.results)



# revision 5
# speedup vs baseline: 1.3396x; 1.3396x over previous
"""YOLO-v1-style loss on 8 Trainium2 NeuronCores (Bass/Tile).

Data-parallel over batch: each core gets 2048 of 16384 batch elements
(128 partitions x 784 cells), computes per-partition partial sums for
the 5 loss terms on-device; host combines and divides by batch.

All inputs are packed host-side into ONE bf16 tensor of 68 channels per
cell, ordered so every hot device op is a contiguous step-1 bf16 pair
(DVE 2x perf mode). The reference's x/S and w/2 scalings are folded
into the host cast; the box-vs-target-box0 intersection uses the
interval identity  iw = (pw'+tw') - max(|dx'|, |dw'|)  (primes = scaled
units), with max(|a|,|b|) done by the abs_max ALU op in one pass.

Channel map (per cell, bf16):
   0: 7   PB8   = [px0,px1, py0,py1, pw0,pw1, ph0,ph1]   (x,y)/S, (w,h)/2
   8:15   T0d8  = [tx0,tx0, ty0,ty0, tw0,tw0, th0,th0]   target box0, dup'd
  16:23   TB8   = [tx0,tx1, ty0,ty1, tw0,tw1, th0,th1]   both target boxes
  24:25   PC2   = [pc0,pc1]                               pred conf, raw
  26:27   OBJ2  = [obj,obj]                               target conf (0/1)
  28:47   PCL   = pred class (20)
  48:67   TCL   = target class (20)

Self-contained: hardcodes all shapes; only needs numpy + concourse (bass).
"""

import numpy as np
import ml_dtypes

import concourse.bass as bass
import concourse.bacc as bacc
import concourse.tile as tile
import concourse.mybir as mybir
from concourse.bass_utils import run_bass_kernel_spmd

f32 = mybir.dt.float32
bf16 = mybir.dt.bfloat16
Alu = mybir.AluOpType
Act = mybir.ActivationFunctionType

S = 7
BATCH = 16384
NCORES = 8
PER = BATCH // NCORES          # 2048 batch elems per core
P = 128                        # partitions
F = PER * 49 // P              # 784 cells per partition
NCH = 68                       # packed channels per cell
NCHUNK = 4
CH = F // NCHUNK               # cells per partition per chunk
NACC = NCHUNK * 6              # acc columns: xy, whS, whR, obj, noobj, cls

INV_S = 1.0 / S


def _ap(t, offs, dims):
    """Raw AP view of tile t: partition dim kept, free dims = [stride, size]."""
    return bass.AP(tensor=t.tensor, offset=t.offset + offs, ap=[t.ap[0]] + dims)


def build_nc(nchunk=NCHUNK):
    ch = F // nchunk
    nc = bacc.Bacc("TRN2", target_bir_lowering=False, debug=False,
                   num_devices=NCORES)
    inp = nc.dram_tensor("inp", [P, F, NCH], bf16, kind="ExternalInput")
    out = nc.dram_tensor("acc_out", [P, nchunk * 6], f32, kind="ExternalOutput")

    V = nc.vector
    A = nc.scalar
    G = nc.gpsimd

    with tile.TileContext(nc) as tc:
        with (
            tc.tile_pool(name="inp", bufs=2) as ip,
            tc.tile_pool(name="wk", bufs=1) as wk,
            tc.tile_pool(name="scr", bufs=2) as sp,
            tc.tile_pool(name="one", bufs=1) as one,
        ):
            acc = one.tile([P, nchunk * 6], f32)
            V.memset(acc, 0.0)

            for k in range(nchunk):
                c0 = k * ch
                a0 = k * 6

                t = ip.tile([P, ch, NCH], bf16, tag="in")
                nc.sync.dma_start(t, inp.ap()[:, c0:c0 + ch, :])

                PB8 = t[:, :, 0:8]
                T0d = t[:, :, 8:16]
                TB8 = t[:, :, 16:24]
                PC2 = t[:, :, 24:26]
                OBJ2 = t[:, :, 26:28]
                OBJ1 = t[:, :, 26]
                PCL = t[:, :, 28:48]
                TCL = t[:, :, 48:68]

                # ---- class loss: diff on DVE, mask on POOL, sq+reduce on ACT
                dc = wk.tile([P, ch, 20], bf16, tag="dc")
                V.tensor_tensor(dc, PCL, TCL, op=Alu.subtract)
                dm = wk.tile([P, ch, 20], bf16, tag="dm")
                objbc = _ap(t, 26, [[NCH, ch], [0, 20]])
                G.tensor_tensor(dm, dc, objbc, op=Alu.mult)
                A.activation(dm, dm, Act.Square,
                             accum_out=acc[:, a0 + 5:a0 + 6])

                # ---- IoU vs target box0 (interval identity) ----
                dI = wk.tile([P, ch, 8], bf16, tag="dI")
                V.tensor_tensor(dI, PB8, T0d, op=Alu.subtract)
                aI = wk.tile([P, ch, 8], bf16, tag="aI")
                A.activation(aI, dI, Act.Abs)
                m4 = wk.tile([P, ch, 4], bf16, tag="m4")
                V.tensor_tensor(m4, aI[:, :, 0:4], aI[:, :, 4:8],
                                op=Alu.max)
                s4 = wk.tile([P, ch, 4], bf16, tag="s4")
                V.tensor_tensor(s4, PB8[:, :, 4:8], T0d[:, :, 4:8], op=Alu.add)
                iwh = wk.tile([P, ch, 4], bf16, tag="iwh")
                V.tensor_tensor(iwh, s4, m4, op=Alu.subtract)
                A.activation(iwh, iwh, Act.Relu)
                inter = wk.tile([P, ch, 2], bf16, tag="inter")
                V.tensor_tensor(inter, iwh[:, :, 0:2], iwh[:, :, 2:4],
                                op=Alu.mult)
                ap2 = wk.tile([P, ch, 2], bf16, tag="ap2")
                V.scalar_tensor_tensor(ap2, PB8[:, :, 4:6], 4.0,
                                       PB8[:, :, 6:8],
                                       op0=Alu.mult, op1=Alu.mult)
                at2 = wk.tile([P, ch, 2], bf16, tag="at2")
                V.scalar_tensor_tensor(at2, T0d[:, :, 4:6], 4.0,
                                       T0d[:, :, 6:8],
                                       op0=Alu.mult, op1=Alu.mult)
                den = wk.tile([P, ch, 2], bf16, tag="den")
                V.tensor_tensor(den, ap2, at2, op=Alu.add)
                V.tensor_tensor(den, den, inter, op=Alu.subtract)
                denf = wk.tile([P, ch, 2], f32, tag="denf")
                V.tensor_copy(denf, den)
                rden = wk.tile([P, ch, 2], f32, tag="rden")
                V.reciprocal_approx_fast(rden, denf)
                iou = wk.tile([P, ch, 2], bf16, tag="iou")
                V.tensor_tensor(iou, inter, rden, op=Alu.mult)

                # ---- responsibility ----
                ge = wk.tile([P, ch], bf16, tag="ge")
                V.tensor_tensor(ge, iou[:, :, 0], iou[:, :, 1], op=Alu.is_ge)
                # resp4 = [r0,r1,r0,r1] per cell (stt needs <=2 free dims,
                # so the pair is materialized twice instead of broadcast)
                resp4 = wk.tile([P, ch, 4], bf16, tag="resp4")
                resp = resp4[:, :, 0:2]
                V.tensor_tensor(resp4[:, :, 0], ge, OBJ1, op=Alu.mult)
                V.tensor_tensor(resp4[:, :, 1], OBJ1, resp4[:, :, 0],
                                op=Alu.subtract)
                V.tensor_copy(resp4[:, :, 2:4], resp)

                # ---- xy loss ----
                dL = wk.tile([P, ch, 4], bf16, tag="dL")
                V.tensor_tensor(dL, PB8[:, :, 0:4], TB8[:, :, 0:4],
                                op=Alu.subtract)
                sqL = wk.tile([P, ch, 4], bf16, tag="sqL")
                A.activation(sqL, dL, Act.Square)
                scr = sp.tile([P, ch, 4], bf16, tag="scr")
                V.scalar_tensor_tensor(scr, sqL, 0.0, resp4,
                                       op0=Alu.bypass, op1=Alu.mult,
                                       accum_out=acc[:, a0 + 0:a0 + 1])

                # ---- wh loss: sum(resp*(pw'+tw')) and sum(resp*sqrt(pw'tw'))
                swh = wk.tile([P, ch, 4], bf16, tag="swh")
                V.tensor_tensor(swh, PB8[:, :, 4:8], TB8[:, :, 4:8],
                                op=Alu.add)
                qwh = wk.tile([P, ch, 4], bf16, tag="qwh")
                V.tensor_tensor(qwh, PB8[:, :, 4:8], TB8[:, :, 4:8],
                                op=Alu.mult)
                r4 = wk.tile([P, ch, 4], bf16, tag="r4")
                A.activation(r4, qwh, Act.Sqrt)
                scr = sp.tile([P, ch, 4], bf16, tag="scr")
                V.scalar_tensor_tensor(scr, swh, 0.0, resp4,
                                       op0=Alu.bypass, op1=Alu.mult,
                                       accum_out=acc[:, a0 + 1:a0 + 2])
                scr = sp.tile([P, ch, 4], bf16, tag="scr")
                V.scalar_tensor_tensor(scr, r4, 0.0, resp4,
                                       op0=Alu.bypass, op1=Alu.mult,
                                       accum_out=acc[:, a0 + 2:a0 + 3])

                # ---- obj loss: resp * (pc - iou)^2 ----
                oc = wk.tile([P, ch, 2], bf16, tag="oc")
                V.tensor_tensor(oc, PC2, iou, op=Alu.subtract)
                A.activation(oc, oc, Act.Square)
                scr = sp.tile([P, ch, 4], bf16, tag="scr")
                V.scalar_tensor_tensor(scr[:, :, 0:2], oc, 0.0, resp,
                                       op0=Alu.bypass, op1=Alu.mult,
                                       accum_out=acc[:, a0 + 3:a0 + 4])

                # ---- noobj loss: (1-obj) * (pc0^2 + pc1^2) ----
                pcsq = wk.tile([P, ch, 2], bf16, tag="pcsq")
                A.activation(pcsq, PC2, Act.Square)
                nm = wk.tile([P, ch, 2], bf16, tag="nm")
                A.activation(nm, OBJ2, Act.Copy, scale=-1.0, bias=1.0)
                scr = sp.tile([P, ch, 4], bf16, tag="scr")
                V.scalar_tensor_tensor(scr[:, :, 0:2], pcsq, 0.0, nm,
                                       op0=Alu.bypass, op1=Alu.mult,
                                       accum_out=acc[:, a0 + 4:a0 + 5])

            nc.sync.dma_start(out.ap(), acc)

    nc.compile()
    return nc


_NC_CACHE = None


def _get_nc():
    global _NC_CACHE
    if _NC_CACHE is None:
        _NC_CACHE = build_nc()
    return _NC_CACHE


def shard_inputs(pred_tensor, target_tensor):
    """Full [16384,7,7,30] f32 inputs -> per-core packed bf16 [P, F, 68]."""
    p = np.ascontiguousarray(pred_tensor, dtype=np.float32)
    p = p.reshape(NCORES, P, F, 30)
    t = np.ascontiguousarray(target_tensor, dtype=np.float32)
    t = t.reshape(NCORES, P, F, 30)

    u = np.empty((NCORES, P, F, NCH), dtype=np.float32)
    # pred boxes, channel-major box-minor, scaled
    u[..., 0] = p[..., 0] * INV_S
    u[..., 1] = p[..., 5] * INV_S
    u[..., 2] = p[..., 1] * INV_S
    u[..., 3] = p[..., 6] * INV_S
    u[..., 4] = p[..., 2] * 0.5
    u[..., 5] = p[..., 7] * 0.5
    u[..., 6] = p[..., 3] * 0.5
    u[..., 7] = p[..., 8] * 0.5
    # target box0 duplicated
    u[..., 8] = u[..., 9] = t[..., 0] * INV_S
    u[..., 10] = u[..., 11] = t[..., 1] * INV_S
    u[..., 12] = u[..., 13] = t[..., 2] * 0.5
    u[..., 14] = u[..., 15] = t[..., 3] * 0.5
    # both target boxes
    u[..., 16] = t[..., 0] * INV_S
    u[..., 17] = t[..., 5] * INV_S
    u[..., 18] = t[..., 1] * INV_S
    u[..., 19] = t[..., 6] * INV_S
    u[..., 20] = t[..., 2] * 0.5
    u[..., 21] = t[..., 7] * 0.5
    u[..., 22] = t[..., 3] * 0.5
    u[..., 23] = t[..., 8] * 0.5
    # confs
    u[..., 24] = p[..., 4]
    u[..., 25] = p[..., 9]
    u[..., 26] = u[..., 27] = t[..., 4]
    # classes
    u[..., 28:48] = p[..., 10:30]
    u[..., 48:68] = t[..., 10:30]

    ub = u.astype(ml_dtypes.bfloat16)
    return [{"inp": ub[c]} for c in range(NCORES)]


def combine(results, nchunk=NCHUNK):
    """Per-core acc_out [P, nchunk*6] -> 5-tuple of loss scalars."""
    cols = np.zeros(6, dtype=np.float64)
    for r in results:
        a = r["acc_out"].astype(np.float64).sum(axis=0)   # [nchunk*6]
        cols += a.reshape(nchunk, 6).sum(axis=0)
    xy, whS, whR, obj, noobj, cls_ = cols
    lxy = (S * S) * xy / BATCH
    lwh = (2.0 * whS - 4.0 * whR) / BATCH
    lobj = obj / BATCH
    lnoobj = noobj / BATCH
    lcls = cls_ / BATCH
    return tuple(np.float32(v) for v in (lxy, lwh, lobj, lnoobj, lcls))


def kernel(pred_tensor, target_tensor):
    nc = _get_nc()
    in_maps = shard_inputs(pred_tensor, target_tensor)
    res = run_bass_kernel_spmd(nc, in_maps, core_ids=list(range(NCORES)))
    return combine(res.results)


# revision 6
# speedup vs baseline: 1.7153x; 1.2804x over previous
"""YOLO-v1-style loss on 8 Trainium2 NeuronCores (Bass/Tile).

Data-parallel over batch: each core gets 2048 of 16384 batch elements
(128 partitions x 784 cells), computes per-partition partial sums for
the 5 loss terms on-device; host combines and divides by batch.

v2 design notes:
- Everything bf16 on DVE/ACT only. GPSIMD is NOT used: its SBUF port is
  shared with the vector engine, so gpsimd tensor_tensor freezes DVE
  ops for its whole duration (measured v1: 7.2us stalls per chunk).
- All tensors are packed host-side PLANE-MAJOR so that every vector op
  reads/writes fully dense step-1 bf16 runs (DVE 2x perf mode):
    pb [P,4,F,2]  planes [px-pair, py-pair, pw-pair, ph-pair] (x/S, w/2)
    t0 [P,4,F,2]  target box0 duplicated: [tx0,tx0],[ty0,ty0],...
    tb [P,4,F,2]  both target boxes: [tx0,tx1],[ty0,ty1],...
    pc [P,2,F,2]  plane0 [pc0,pc1] raw conf, plane1 [obj,obj]
    cl [P,2,F,20] plane0 pred classes, plane1 target classes
- IoU via the interval identity iw = (pw'+tw') - max(|dx'|,|dw'|) in
  scaled units (x/S, w/2); areas rescaled by 4 inside an stt.
- Class loss: diff + square, then a pairwise fold tree 20->10->4->2 so
  the obj mask is applied against the dense [obj,obj] pair (no 1x
  broadcast reads anywhere).
- loss_obj uses (pc_b - iou_b)^2 masked by resp_b, which equals the
  reference's (pc_b - max_iou)^2 since resp selects the argmax box.

Self-contained: hardcodes all shapes; only needs numpy + concourse.
"""

import numpy as np
import ml_dtypes

import concourse.bass as bass
import concourse.bacc as bacc
import concourse.tile as tile
import concourse.mybir as mybir
from concourse.bass_utils import run_bass_kernel_spmd

f32 = mybir.dt.float32
bf16 = mybir.dt.bfloat16
Alu = mybir.AluOpType
Act = mybir.ActivationFunctionType

S = 7
BATCH = 16384
NCORES = 8
PER = BATCH // NCORES          # 2048 batch elems per core
P = 128                        # partitions
F = PER * 49 // P              # 784 cells per partition
NCHUNK = 2
NCOL = 7                       # acc cols: XY, WS, WR, OBJ, PSUM, QSUM, CLS

INV_S = 1.0 / S


def build_nc(nchunk=NCHUNK):
    ch = F // nchunk
    nc = bacc.Bacc("TRN2", target_bir_lowering=False, debug=False,
                   num_devices=NCORES)
    pb_d = nc.dram_tensor("pb", [P, 4, F, 2], bf16, kind="ExternalInput")
    t0_d = nc.dram_tensor("t0", [P, 4, F, 2], bf16, kind="ExternalInput")
    tb_d = nc.dram_tensor("tb", [P, 4, F, 2], bf16, kind="ExternalInput")
    pc_d = nc.dram_tensor("pc", [P, 2, F, 2], bf16, kind="ExternalInput")
    cl_d = nc.dram_tensor("cl", [P, 2, F, 20], bf16, kind="ExternalInput")
    out = nc.dram_tensor("acc_out", [P, nchunk * NCOL], f32,
                         kind="ExternalOutput")

    V = nc.vector
    A = nc.scalar

    with tile.TileContext(nc) as tc:
        with (
            tc.tile_pool(name="inp", bufs=2) as ip,
            tc.tile_pool(name="wk", bufs=1) as wk,
            tc.tile_pool(name="scr", bufs=2) as sp,
            tc.tile_pool(name="one", bufs=1) as one,
        ):
            acc = one.tile([P, nchunk * NCOL], f32)
            V.memset(acc, 0.0)

            for k in range(nchunk):
                c0 = k * ch
                a0 = k * NCOL

                pbt = ip.tile([P, 4, ch, 2], bf16, tag="pb")
                nc.sync.dma_start(pbt, pb_d.ap()[:, :, c0:c0 + ch, :])
                t0t = ip.tile([P, 4, ch, 2], bf16, tag="t0")
                nc.sync.dma_start(t0t, t0_d.ap()[:, :, c0:c0 + ch, :])
                tbt = ip.tile([P, 4, ch, 2], bf16, tag="tb")
                nc.sync.dma_start(tbt, tb_d.ap()[:, :, c0:c0 + ch, :])
                pct = ip.tile([P, 2, ch, 2], bf16, tag="pc")
                nc.sync.dma_start(pct, pc_d.ap()[:, :, c0:c0 + ch, :])
                clt = ip.tile([P, 2, ch, 20], bf16, tag="cl")
                nc.sync.dma_start(clt, cl_d.ap()[:, :, c0:c0 + ch, :])

                PC2 = pct[:, 0]          # [P,ch,2] pred conf pair
                OBJ2 = pct[:, 1]         # [P,ch,2] obj duplicated

                # --- DMA-only-dependent DVE ops first (overlap w/ ACT) ---
                dc = wk.tile([P, ch, 20], bf16, tag="dc")
                V.tensor_tensor(dc, clt[:, 0], clt[:, 1], op=Alu.subtract)
                dI = wk.tile([P, 4, ch, 2], bf16, tag="dI")
                V.tensor_tensor(dI, pbt, t0t, op=Alu.subtract)
                s2 = wk.tile([P, 2, ch, 2], bf16, tag="s2")
                V.tensor_tensor(s2, pbt[:, 2:4], t0t[:, 2:4], op=Alu.add)
                ap2 = wk.tile([P, ch, 2], bf16, tag="ap2")
                V.scalar_tensor_tensor(ap2, pbt[:, 2], 4.0, pbt[:, 3],
                                       op0=Alu.mult, op1=Alu.mult)
                at2 = wk.tile([P, ch, 2], bf16, tag="at2")
                V.scalar_tensor_tensor(at2, t0t[:, 2], 4.0, t0t[:, 3],
                                       op0=Alu.mult, op1=Alu.mult)
                dL = wk.tile([P, 2, ch, 2], bf16, tag="dL")
                V.tensor_tensor(dL, pbt[:, 0:2], tbt[:, 0:2],
                                op=Alu.subtract)
                swh = wk.tile([P, 2, ch, 2], bf16, tag="swh")
                V.tensor_tensor(swh, pbt[:, 2:4], tbt[:, 2:4], op=Alu.add)
                qwh = wk.tile([P, 2, ch, 2], bf16, tag="qwh")
                V.tensor_tensor(qwh, pbt[:, 2:4], tbt[:, 2:4], op=Alu.mult)

                # --- ACT: squares/abs/sqrt (parallel with DVE) ---
                A.activation(dc, dc, Act.Square)          # in-place dcsq
                A.activation(dI, dI, Act.Abs)             # in-place |dI|
                sqL = wk.tile([P, 2, ch, 2], bf16, tag="sqL")
                A.activation(sqL, dL, Act.Square)
                A.activation(qwh, qwh, Act.Sqrt)          # in-place r
                pcsq = wk.tile([P, ch, 2], bf16, tag="pcsq")
                A.activation(pcsq, PC2, Act.Square,
                             accum_out=acc[:, a0 + 4:a0 + 5])   # sum pc^2

                # --- IoU ---
                m2 = wk.tile([P, 2, ch, 2], bf16, tag="m2")
                V.tensor_tensor(m2, dI[:, 0:2], dI[:, 2:4], op=Alu.max)
                V.tensor_tensor(s2, s2, m2, op=Alu.subtract)  # iwh in-place
                A.activation(s2, s2, Act.Relu)
                inter = wk.tile([P, ch, 2], bf16, tag="inter")
                V.tensor_tensor(inter, s2[:, 0], s2[:, 1], op=Alu.mult)
                V.tensor_tensor(ap2, ap2, at2, op=Alu.add)    # den in-place
                denf = wk.tile([P, ch, 2], f32, tag="denf")
                V.scalar_tensor_tensor(denf, ap2, 0.0, inter,
                                       op0=Alu.bypass, op1=Alu.subtract)
                rden = wk.tile([P, ch, 2], f32, tag="rden")
                V.reciprocal_approx_fast(rden, denf)
                iou = wk.tile([P, ch, 2], bf16, tag="iou")
                V.tensor_tensor(iou, inter, rden, op=Alu.mult)

                # --- responsibility (resp duplicated across both planes) ---
                ge = wk.tile([P, ch], bf16, tag="ge")
                V.tensor_tensor(ge, iou[:, :, 0], iou[:, :, 1], op=Alu.is_ge)
                resp8 = wk.tile([P, 2, ch, 2], bf16, tag="resp8")
                V.tensor_tensor(resp8[:, 0, :, 0], ge, pct[:, 1, :, 0],
                                op=Alu.mult)
                V.tensor_tensor(resp8[:, 0, :, 1], pct[:, 1, :, 1],
                                resp8[:, 0, :, 0], op=Alu.subtract)
                V.tensor_copy(resp8[:, 1], resp8[:, 0])
                resp2 = resp8[:, 0]

                # --- masked accumulations ---
                scr4 = sp.tile([P, 2, ch, 2], bf16, tag="scr4")
                V.scalar_tensor_tensor(scr4, sqL, 0.0, resp8,
                                       op0=Alu.bypass, op1=Alu.mult,
                                       accum_out=acc[:, a0 + 0:a0 + 1])
                scr4 = sp.tile([P, 2, ch, 2], bf16, tag="scr4")
                V.scalar_tensor_tensor(scr4, swh, 0.0, resp8,
                                       op0=Alu.bypass, op1=Alu.mult,
                                       accum_out=acc[:, a0 + 1:a0 + 2])
                scr4 = sp.tile([P, 2, ch, 2], bf16, tag="scr4")
                V.scalar_tensor_tensor(scr4, qwh, 0.0, resp8,
                                       op0=Alu.bypass, op1=Alu.mult,
                                       accum_out=acc[:, a0 + 2:a0 + 3])

                oc = wk.tile([P, ch, 2], bf16, tag="oc")
                V.tensor_tensor(oc, PC2, iou, op=Alu.subtract)
                A.activation(oc, oc, Act.Square)
                scr4 = sp.tile([P, 2, ch, 2], bf16, tag="scr4")
                V.scalar_tensor_tensor(scr4[:, 0], oc, 0.0, resp2,
                                       op0=Alu.bypass, op1=Alu.mult,
                                       accum_out=acc[:, a0 + 3:a0 + 4])
                scr4 = sp.tile([P, 2, ch, 2], bf16, tag="scr4")
                V.scalar_tensor_tensor(scr4[:, 0], pcsq, 0.0, OBJ2,
                                       op0=Alu.bypass, op1=Alu.mult,
                                       accum_out=acc[:, a0 + 5:a0 + 6])

                # --- class fold tree: 20 -> 10 -> 4+2 -> 2, then obj mask ---
                u10 = wk.tile([P, ch, 10], bf16, tag="u10")
                V.tensor_tensor(u10, dc[:, :, 0:10], dc[:, :, 10:20],
                                op=Alu.add)
                w4 = wk.tile([P, ch, 4], bf16, tag="w4")
                V.tensor_tensor(w4, u10[:, :, 0:4], u10[:, :, 4:8],
                                op=Alu.add)
                a2 = wk.tile([P, ch, 2], bf16, tag="a2")
                V.tensor_tensor(a2, w4[:, :, 0:2], w4[:, :, 2:4], op=Alu.add)
                V.tensor_tensor(a2, a2, u10[:, :, 8:10], op=Alu.add)
                scr4 = sp.tile([P, 2, ch, 2], bf16, tag="scr4")
                V.scalar_tensor_tensor(scr4[:, 0], a2, 0.0, OBJ2,
                                       op0=Alu.bypass, op1=Alu.mult,
                                       accum_out=acc[:, a0 + 6:a0 + 7])

            nc.sync.dma_start(out.ap(), acc)

    nc.compile()
    return nc


_NC_CACHE = None


def _get_nc():
    global _NC_CACHE
    if _NC_CACHE is None:
        _NC_CACHE = build_nc()
    return _NC_CACHE


def shard_inputs(pred_tensor, target_tensor):
    """Full [16384,7,7,30] f32 -> per-core plane-major bf16 tensors."""
    p = np.ascontiguousarray(pred_tensor, dtype=np.float32)
    p = p.reshape(NCORES, P, F, 30)
    t = np.ascontiguousarray(target_tensor, dtype=np.float32)
    t = t.reshape(NCORES, P, F, 30)

    bf = ml_dtypes.bfloat16
    pb = np.empty((NCORES, P, 4, F, 2), dtype=np.float32)
    t0 = np.empty_like(pb)
    tb = np.empty_like(pb)
    for ax, (c0, c1, sc) in enumerate(
            [(0, 5, INV_S), (1, 6, INV_S), (2, 7, 0.5), (3, 8, 0.5)]):
        pb[:, :, ax, :, 0] = p[..., c0] * sc
        pb[:, :, ax, :, 1] = p[..., c1] * sc
        t0[:, :, ax, :, 0] = t0[:, :, ax, :, 1] = t[..., c0] * sc
        tb[:, :, ax, :, 0] = t[..., c0] * sc
        tb[:, :, ax, :, 1] = t[..., c1] * sc
    pc = np.empty((NCORES, P, 2, F, 2), dtype=np.float32)
    pc[:, :, 0, :, 0] = p[..., 4]
    pc[:, :, 0, :, 1] = p[..., 9]
    pc[:, :, 1, :, 0] = pc[:, :, 1, :, 1] = t[..., 4]
    cl = np.empty((NCORES, P, 2, F, 20), dtype=np.float32)
    cl[:, :, 0] = p[..., 10:30]
    cl[:, :, 1] = t[..., 10:30]

    pb = pb.astype(bf); t0 = t0.astype(bf); tb = tb.astype(bf)
    pc = pc.astype(bf); cl = cl.astype(bf)
    return [{"pb": pb[c], "t0": t0[c], "tb": tb[c], "pc": pc[c],
             "cl": cl[c]} for c in range(NCORES)]


def combine(results, nchunk=NCHUNK):
    """Per-core acc_out [P, nchunk*NCOL] -> 5-tuple of loss scalars."""
    cols = np.zeros(NCOL, dtype=np.float64)
    for r in results:
        a = r["acc_out"].astype(np.float64).sum(axis=0)
        cols += a.reshape(nchunk, NCOL).sum(axis=0)
    xy, ws, wr, obj, psum, qsum, cls_ = cols
    lxy = (S * S) * xy / BATCH
    lwh = (2.0 * ws - 4.0 * wr) / BATCH
    lobj = obj / BATCH
    lnoobj = (psum - qsum) / BATCH
    lcls = cls_ / BATCH
    return tuple(np.float32(v) for v in (lxy, lwh, lobj, lnoobj, lcls))


def kernel(pred_tensor, target_tensor):
    nc = _get_nc()
    in_maps = shard_inputs(pred_tensor, target_tensor)
    res = run_bass_kernel_spmd(nc, in_maps, core_ids=list(range(NCORES)))
    return combine(res.results)


# revision 11
# speedup vs baseline: 1.8175x; 1.0596x over previous
"""YOLO-v1-style loss on 8 Trainium2 NeuronCores (Bass/Tile).

Data-parallel over batch: each core gets 2048 of 16384 batch elements
(128 partitions x 784 cells), computes per-partition partial sums for
the 5 loss terms on-device; host combines and divides by batch.

v2 design notes:
- Everything bf16 on DVE/ACT only. GPSIMD is NOT used: its SBUF port is
  shared with the vector engine, so gpsimd tensor_tensor freezes DVE
  ops for its whole duration (measured v1: 7.2us stalls per chunk).
- All tensors are packed host-side PLANE-MAJOR so that every vector op
  reads/writes fully dense step-1 bf16 runs (DVE 2x perf mode):
    pb [P,4,F,2]  planes [px-pair, py-pair, pw-pair, ph-pair] (x/S, w/2)
    t0 [P,4,F,2]  target box0 duplicated: [tx0,tx0],[ty0,ty0],...
    tb [P,4,F,2]  both target boxes: [tx0,tx1],[ty0,ty1],...
    pc [P,2,F,2]  plane0 [pc0,pc1] raw conf, plane1 [obj,obj]
    cl [P,2,F,20] plane0 pred classes, plane1 target classes
- IoU via the interval identity iw = (pw'+tw') - max(|dx'|,|dw'|) in
  scaled units (x/S, w/2); areas rescaled by 4 inside an stt.
- Class loss: diff + square, then a pairwise fold tree 20->10->4->2 so
  the obj mask is applied against the dense [obj,obj] pair (no 1x
  broadcast reads anywhere).
- loss_obj uses (pc_b - iou_b)^2 masked by resp_b, which equals the
  reference's (pc_b - max_iou)^2 since resp selects the argmax box.

Self-contained: hardcodes all shapes; only needs numpy + concourse.
"""

import numpy as np
import ml_dtypes

import concourse.bass as bass
import concourse.bacc as bacc
import concourse.tile as tile
import concourse.mybir as mybir
from concourse.bass_utils import run_bass_kernel_spmd

f32 = mybir.dt.float32
bf16 = mybir.dt.bfloat16
Alu = mybir.AluOpType
Act = mybir.ActivationFunctionType

S = 7
BATCH = 16384
NCORES = 8
PER = BATCH // NCORES          # 2048 batch elems per core
P = 128                        # partitions
F = PER * 49 // P              # 784 cells per partition
CHUNKS = [112, 336, 336]       # uneven (bisect step 2)
NCHUNK = len(CHUNKS)
NCOL = 7                       # acc cols: XY, WS, WR, OBJ, PSUM, QSUM, CLS

INV_S = 1.0 / S


def build_nc(chunks=tuple(CHUNKS)):
    nchunk = len(chunks)
    nc = bacc.Bacc("TRN2", target_bir_lowering=False, debug=False,
                   num_devices=NCORES)
    geo_d = nc.dram_tensor("geo", [P, 14, F, 2], bf16, kind="ExternalInput")
    cl_d = nc.dram_tensor("cl", [P, 2, F, 20], bf16, kind="ExternalInput")
    out = nc.dram_tensor("acc_out", [P, nchunk * NCOL], f32,
                         kind="ExternalOutput")

    V = nc.vector
    A = nc.scalar

    with tile.TileContext(nc) as tc:
        with (
            tc.tile_pool(name="inp", bufs=2) as ip,
            tc.tile_pool(name="wk", bufs=1) as wk,
            tc.tile_pool(name="scr", bufs=2) as sp,
            tc.tile_pool(name="one", bufs=1) as one,
        ):
            acc = one.tile([P, nchunk * NCOL], f32)
            V.memset(acc, 0.0)

            c0 = 0
            for k, ch in enumerate(chunks):
                a0 = k * NCOL

                geot = ip.tile([P, 14, ch, 2], bf16, tag="geo")
                nc.sync.dma_start(geot, geo_d.ap()[:, :, c0:c0 + ch, :])
                clt = ip.tile([P, 2, ch, 20], bf16, tag="cl")
                nc.sync.dma_start(clt, cl_d.ap()[:, :, c0:c0 + ch, :])

                pbt = geot[:, 0:4]
                t0t = geot[:, 4:8]
                tbt = geot[:, 8:12]
                PC2 = geot[:, 12]        # [P,ch,2] pred conf pair
                OBJ2 = geot[:, 13]       # [P,ch,2] obj duplicated

                # --- DMA-only-dependent DVE ops first (overlap w/ ACT) ---
                dc = wk.tile([P, ch, 20], bf16, tag="dc")
                V.tensor_tensor(dc, clt[:, 0], clt[:, 1], op=Alu.subtract)
                dI = wk.tile([P, 4, ch, 2], bf16, tag="dI")
                V.tensor_tensor(dI, pbt, t0t, op=Alu.subtract)
                s2 = wk.tile([P, 2, ch, 2], bf16, tag="s2")
                V.tensor_tensor(s2, pbt[:, 2:4], t0t[:, 2:4], op=Alu.add)
                ap2 = wk.tile([P, ch, 2], bf16, tag="ap2")
                V.tensor_tensor(ap2, pbt[:, 2], pbt[:, 3], op=Alu.mult)
                at2 = wk.tile([P, ch, 2], bf16, tag="at2")
                V.tensor_tensor(at2, t0t[:, 2], t0t[:, 3], op=Alu.mult)
                dL = wk.tile([P, 2, ch, 2], bf16, tag="dL")
                V.tensor_tensor(dL, pbt[:, 0:2], tbt[:, 0:2],
                                op=Alu.subtract)
                swh = wk.tile([P, 2, ch, 2], bf16, tag="swh")
                V.tensor_tensor(swh, pbt[:, 2:4], tbt[:, 2:4], op=Alu.add)
                qwh = wk.tile([P, 2, ch, 2], bf16, tag="qwh")
                V.tensor_tensor(qwh, pbt[:, 2:4], tbt[:, 2:4], op=Alu.mult)

                # --- ACT: abs/squares/sqrt (parallel with DVE) ---
                A.activation(dI, dI, Act.Abs)             # in-place |dI|
                A.activation(qwh, qwh, Act.Sqrt)          # in-place r
                pcsq = wk.tile([P, ch, 2], bf16, tag="pcsq")
                A.activation(pcsq, PC2, Act.Square,
                             accum_out=acc[:, a0 + 4:a0 + 5])   # sum pc^2

                # --- IoU ---
                m2 = wk.tile([P, 2, ch, 2], bf16, tag="m2")
                V.tensor_tensor(m2, dI[:, 0:2], dI[:, 2:4], op=Alu.max)
                V.tensor_tensor(s2, s2, m2, op=Alu.subtract)  # iwh in-place
                A.activation(s2, s2, Act.Relu)
                inter = wk.tile([P, ch, 2], bf16, tag="inter")
                V.tensor_tensor(inter, s2[:, 0], s2[:, 1], op=Alu.mult)
                V.tensor_tensor(ap2, ap2, at2, op=Alu.add)    # den in-place
                denf = wk.tile([P, ch, 2], f32, tag="denf")
                V.scalar_tensor_tensor(denf, ap2, 4.0, inter,
                                       op0=Alu.mult, op1=Alu.subtract)
                rden = wk.tile([P, ch, 2], f32, tag="rden")
                V.reciprocal_approx_fast(rden, denf)
                iou = wk.tile([P, ch, 2], bf16, tag="iou")
                V.tensor_tensor(iou, inter, rden, op=Alu.mult)

                # --- responsibility (resp duplicated across both planes) ---
                ge = wk.tile([P, ch], bf16, tag="ge")
                V.tensor_tensor(ge, iou[:, :, 0], iou[:, :, 1], op=Alu.is_ge)
                resp8 = wk.tile([P, 2, ch, 2], bf16, tag="resp8")
                V.tensor_tensor(resp8[:, 0, :, 0], ge, geot[:, 13, :, 0],
                                op=Alu.mult)
                V.tensor_tensor(resp8[:, 0, :, 1], geot[:, 13, :, 1],
                                resp8[:, 0, :, 0], op=Alu.subtract)
                V.tensor_copy(resp8[:, 1], resp8[:, 0])
                resp2 = resp8[:, 0]

                # --- masked accumulations: mask at 2x on DVE (resp^2=resp,
                # obj^2=obj since masks are 0/1), reduce on ACT accum ---
                dLm = wk.tile([P, 2, ch, 2], bf16, tag="dLm")
                V.tensor_tensor(dLm, dL, resp8, op=Alu.mult)
                A.activation(dLm, dLm, Act.Square,
                             accum_out=acc[:, a0 + 0:a0 + 1])
                swhm = wk.tile([P, 2, ch, 2], bf16, tag="swhm")
                V.tensor_tensor(swhm, swh, resp8, op=Alu.mult)
                A.activation(swhm, swhm, Act.Copy,
                             accum_out=acc[:, a0 + 1:a0 + 2])
                rm = wk.tile([P, 2, ch, 2], bf16, tag="rm")
                V.tensor_tensor(rm, qwh, resp8, op=Alu.mult)
                A.activation(rm, rm, Act.Copy,
                             accum_out=acc[:, a0 + 2:a0 + 3])

                oc = wk.tile([P, ch, 2], bf16, tag="oc")
                V.tensor_tensor(oc, PC2, iou, op=Alu.subtract)
                ocm = wk.tile([P, ch, 2], bf16, tag="ocm")
                V.tensor_tensor(ocm, oc, resp2, op=Alu.mult)
                A.activation(ocm, ocm, Act.Square,
                             accum_out=acc[:, a0 + 3:a0 + 4])
                qm = wk.tile([P, ch, 2], bf16, tag="qm")
                V.tensor_tensor(qm, PC2, OBJ2, op=Alu.mult)
                A.activation(qm, qm, Act.Square,
                             accum_out=acc[:, a0 + 5:a0 + 6])

                # --- class fold tree: 20 -> 10 -> 4+2 -> 2, then obj mask ---
                A.activation(dc, dc, Act.Square)          # in-place dcsq
                u10 = wk.tile([P, ch, 10], bf16, tag="u10")
                V.tensor_tensor(u10, dc[:, :, 0:10], dc[:, :, 10:20],
                                op=Alu.add)
                w4 = wk.tile([P, ch, 4], bf16, tag="w4")
                V.tensor_tensor(w4, u10[:, :, 0:4], u10[:, :, 4:8],
                                op=Alu.add)
                a2 = wk.tile([P, ch, 2], bf16, tag="a2")
                V.tensor_tensor(a2, w4[:, :, 0:2], w4[:, :, 2:4], op=Alu.add)
                V.tensor_tensor(a2, a2, u10[:, :, 8:10], op=Alu.add)
                a2m = wk.tile([P, ch, 2], bf16, tag="a2m")
                V.tensor_tensor(a2m, a2, OBJ2, op=Alu.mult)
                A.activation(a2m, a2m, Act.Copy,
                             accum_out=acc[:, a0 + 6:a0 + 7])
                c0 += ch

            nc.sync.dma_start(out.ap(), acc)

    nc.compile()
    return nc


_NC_CACHE = None


def _get_nc():
    global _NC_CACHE
    if _NC_CACHE is None:
        _NC_CACHE = build_nc()
    return _NC_CACHE


def shard_inputs(pred_tensor, target_tensor):
    """Full [16384,7,7,30] f32 -> per-core plane-major bf16 tensors."""
    p = np.ascontiguousarray(pred_tensor, dtype=np.float32)
    p = p.reshape(NCORES, P, F, 30)
    t = np.ascontiguousarray(target_tensor, dtype=np.float32)
    t = t.reshape(NCORES, P, F, 30)

    bf = ml_dtypes.bfloat16
    geo = np.empty((NCORES, P, 14, F, 2), dtype=np.float32)
    for ax, (c0, c1, sc) in enumerate(
            [(0, 5, INV_S), (1, 6, INV_S), (2, 7, 0.5), (3, 8, 0.5)]):
        geo[:, :, ax, :, 0] = p[..., c0] * sc          # pb planes 0-3
        geo[:, :, ax, :, 1] = p[..., c1] * sc
        geo[:, :, 4 + ax, :, 0] = t[..., c0] * sc      # t0 planes 4-7 (dup)
        geo[:, :, 4 + ax, :, 1] = t[..., c0] * sc
        geo[:, :, 8 + ax, :, 0] = t[..., c0] * sc      # tb planes 8-11
        geo[:, :, 8 + ax, :, 1] = t[..., c1] * sc
    geo[:, :, 12, :, 0] = p[..., 4]                    # pred conf pair
    geo[:, :, 12, :, 1] = p[..., 9]
    geo[:, :, 13, :, 0] = geo[:, :, 13, :, 1] = t[..., 4]   # obj pair
    cl = np.empty((NCORES, P, 2, F, 20), dtype=np.float32)
    cl[:, :, 0] = p[..., 10:30]
    cl[:, :, 1] = t[..., 10:30]

    geo = geo.astype(bf); cl = cl.astype(bf)
    return [{"geo": geo[c], "cl": cl[c]} for c in range(NCORES)]


def combine(results, nchunk=NCHUNK):
    """Per-core acc_out [P, nchunk*NCOL] -> 5-tuple of loss scalars."""
    cols = np.zeros(NCOL, dtype=np.float64)
    for r in results:
        a = r["acc_out"].astype(np.float64).sum(axis=0)
        cols += a.reshape(nchunk, NCOL).sum(axis=0)
    xy, ws, wr, obj, psum, qsum, cls_ = cols
    lxy = (S * S) * xy / BATCH
    lwh = (2.0 * ws - 4.0 * wr) / BATCH
    lobj = obj / BATCH
    lnoobj = (psum - qsum) / BATCH
    lcls = cls_ / BATCH
    return tuple(np.float32(v) for v in (lxy, lwh, lobj, lnoobj, lcls))


def kernel(pred_tensor, target_tensor):
    nc = _get_nc()
    in_maps = shard_inputs(pred_tensor, target_tensor)
    res = run_bass_kernel_spmd(nc, in_maps, core_ids=list(range(NCORES)))
    return combine(res.results)
